# revision 43
# baseline (speedup 1.0000x reference)
"""ChannelAwareAttentionModule TRN2 kernel (v12: cross-sample PE interleave).

Math (per sample s; biases are no-ops because InstanceNorm removes them):
  thetaN/phiN/gN = relu(instnorm(w @ x))        [Ci=128, N=4096]
  f = thetaN @ phiN^T; attn = softmax(f, axis=1)
  y = attn @ gN; Z[ci, q*128+r] = y[r, 32*ci+q]
  out = instnorm(W_w @ Z) + x                   [256, 4096]

Precision: fp16 (10 mantissa bits) for x/w/projections/gram inputs -- the
gram itself accumulates fp32 in PSUM so logits (~650 +- 70) stay accurate.

Factorizations (relu(rstd*(x-mean)) = rstd*relu(x-mean), rstd>0):
  theta: apply = (x sub mean) max 0; rstd folds into the softmax Exp scale.
  phi:   apply = (x sub mean) mult rstd; relu moves into the transpose-evict.
  g:     fp16 apply; rstd folds into attn^T rows.
  conv:  recompute + diag(sigma) @ x accumulated in psum makes the final
         evict a 2-op scale+bias that lands instnorm(U) + x exactly.
  rstd = exp(-0.5*ln(var+eps)); the act-table patch keeps ln/exp/relu/
  identity/copy in ONE table (natural_log_exp_and_others) -- no reloads.

Scheduling: the PE HAM clock-gate drops to 1.2 GHz after any ~3.4us
window of low PE activity, and costs ~40us/run when the proj_finish /
softmax windows idle the PE.  Sample 1's projection matmuls are emitted
interleaved with sample 0's finish/attn phases (praw/stat rings sized so
the cross-sample WAR waits never point at later-emitted readers, which
would deadlock the in-order queues).  Projection evicts are pinned to
ACT so DVE stays stats-only (aggregations aren't queued behind evicts);
applies are pinned to DVE (530ns vs 1140ns on ACT, and same-engine as
the aggregation they depend on).  The conv tail pipelines
stats(s,oc)/out(s,oc) so PE recompute overlaps the next stats pass.

Sharding: data-parallel over batch, 2 samples per core, 8 cores.
"""
import sys

sys.path.insert(0, "/opt/trn_rl_repo")

import numpy as np

import concourse.bass as bass
import concourse.bacc as bacc
import concourse.tile as tile
from concourse import mybir
from concourse.bass_utils import run_bass_kernel_spmd
from concourse.masks import make_identity

N_CORES = 8
B, C, CI, H, W = 16, 256, 128, 64, 64
N = H * W  # 4096
B_LOC = B // N_CORES  # 2 samples per core
KCH = C // 128  # 2 contraction chunks of input channels
NT = N // 128  # 32 column tiles
NPAIR = 4  # 4 psum pairs of 1024 f32 per 4096 cols
EPS = 1e-5

F32 = mybir.dt.float32
F16 = mybir.dt.float16

_CACHE = {}


def _patch_act_tables():
    """Steer the ACT-table pass to the one table that serves every function
    this kernel uses (ln, exp, relu, identity, copy, square).

    Without this the pass greedily alternates between `natural_log` (has ln,
    no exp) and `exp_and_others` (has exp, no ln), reloading a 1.3us table
    ~17 times per kernel.  We strip our functions from every other table so
    `natural_log_exp_and_others` is the only candidate; the dict keeps its
    original order/length so act_func_set_id indices stay valid."""
    import concourse.hw_specs as hw_specs

    if getattr(hw_specs.get_activation_tables, "_caam_patched", False):
        return
    real_fn = hw_specs.get_activation_tables
    AF = mybir.ActivationFunctionType
    ours = {AF.Ln, AF.Exp, AF.Identity, AF.Relu, AF.Copy, AF.Square}
    keep = "natural_log_exp_and_others"

    def patched(arch, _real=real_fn, _ours=ours, _keep=keep):
        tables = _real(arch)
        if _keep not in tables:
            return tables
        return {
            name: (set(fns) if name == _keep else set(fns) - _ours)
            for name, fns in tables.items()
        }

    patched._caam_patched = True
    hw_specs.get_activation_tables = patched
    bacc.get_activation_tables = patched
    try:
        import concourse.bass_interp as bass_interp

        bass_interp.get_activation_tables = patched
    except ImportError:
        pass


def build_nc():
    _patch_act_tables()
    nc = bacc.Bacc("TRN2", target_bir_lowering=False)

    x_ext = nc.declare_dram_parameter("x", [B_LOC, C, N], F16, isOutput=False)
    # stacked projection weights, host layout [128, KCH, 3, 128] = [c128, k, proj, ci]
    w_ext = nc.declare_dram_parameter("w3", [128, KCH, 3, CI], F16, isOutput=False)
    ww_ext = nc.declare_dram_parameter("ww", [CI, C], F16, isOutput=False)
    # fp16 output halves the HBM write traffic; the host casts back to f32
    # (instnorm+residual values are O(1), so fp16 adds only ~2e-4 error)
    out_ext = nc.declare_dram_parameter("out", [B_LOC, C, N], F16, isOutput=True)

    AL = mybir.AluOpType
    AF = mybir.ActivationFunctionType

    with tile.TileContext(nc) as tc:
        from contextlib import ExitStack

        with ExitStack() as ctx:
            consts = ctx.enter_context(tc.tile_pool(name="consts", bufs=1))
            xpool = ctx.enter_context(tc.tile_pool(name="xpool", bufs=8))
            rawp = ctx.enter_context(tc.tile_pool(name="rawp", bufs=4))
            grawp = ctx.enter_context(tc.tile_pool(name="grawp", bufs=2))
            pbp = ctx.enter_context(tc.tile_pool(name="pbp", bufs=8))
            gtp = ctx.enter_context(tc.tile_pool(name="gtp", bufs=2))
            tpp = ctx.enter_context(tc.tile_pool(name="tpp", bufs=1))
            zp = ctx.enter_context(tc.tile_pool(name="zp", bufs=2))
            orp = ctx.enter_context(tc.tile_pool(name="orp", bufs=6))
            small = ctx.enter_context(tc.tile_pool(name="small", bufs=2))
            banks = ctx.enter_context(tc.tile_pool(name="banks", bufs=3, space="PSUM"))
            psmall = ctx.enter_context(tc.tile_pool(name="psmall", bufs=1, space="PSUM"))

            # ---- constants ----
            ident32 = consts.tile([128, 128], F32)
            make_identity(nc, ident32[:])
            identh = consts.tile([128, 128], F16)
            nc.vector.tensor_copy(identh[:], ident32[:])
            w_sb = consts.tile([128, KCH, 3, CI], F16)
            nc.scalar.dma_start(w_sb[:], w_ext[:])
            ww_sb = consts.tile([CI, C], F16)
            nc.scalar.dma_start(ww_sb[:], ww_ext[:])
            eps_t = consts.tile([128, 1], F32)
            nc.vector.memset(eps_t[:], EPS)

            # ---- greedy engine-balance ----------------------------------
            # Pool has no PSUM port: PSUM work goes to DVE ("D") or ACT ("A").
            load = {"D": 0.0, "A": 0.0}
            ENG = {"D": nc.vector, "A": nc.scalar}

            def pick(costs):
                k = min(costs, key=lambda e: load[e] + costs[e])
                load[k] += costs[k]
                return k

            # per-op costs (ns) for 1024 cols, fit to ntff measurements
            EV_PROJ = {"D": 1170.0, "A": 1160.0}  # psum f32 -> sbuf f16 cast
            EV_TP = {"D": 660.0, "A": 570.0}      # psum f16 -> sbuf f16 (2x)
            EV_Z = {"D": 1170.0, "A": 1160.0}     # psum f32 -> sbuf f16 cast
            EV_TS = {"D": 1300.0, "A": 1240.0}    # psum f32 scale+bias -> f32
            APPLY_1024 = {"D": 530.0, "A": 1140.0}  # sbuf f16 2-op

            def evict(dst, src, costs, func=None, bias=None, scale=None,
                      eng=None):
                if eng is None:
                    eng = pick(costs)
                else:
                    load[eng] += costs[eng]
                if eng == "A":
                    if func == "relu":
                        nc.scalar.activation(dst, src, AF.Relu)
                    elif func == "ts":
                        nc.scalar.activation(
                            dst, src, AF.Identity, bias=bias, scale=scale)
                    else:
                        nc.scalar.copy(dst, src)
                else:
                    if func == "relu":
                        nc.vector.tensor_scalar_max(dst, src, 0.0)
                    elif func == "ts":
                        nc.vector.tensor_scalar(
                            dst, src, scale, bias, op0=AL.mult, op1=AL.add)
                    else:
                        nc.vector.tensor_copy(dst, src)

            def rstd_chain(mv, tag, want_sigma=False):
                """mv [128,2]=(mean,var) -> rstd=exp(-ln(var+eps)/2) (+sigma)."""
                L = small.tile([128, 1], F32, tag="lnv", name="lnv")
                nc.scalar.activation(L[:], mv[:, 1:2], AF.Ln, bias=eps_t[:], scale=1.0)
                rstd = small.tile([128, 1], F32, tag=f"rstd_{tag}", name="rstd")
                nc.scalar.activation(rstd[:], L[:], AF.Exp, bias=0.0, scale=-0.5)
                if not want_sigma:
                    return rstd, None
                sig = small.tile([128, 1], F32, tag=f"sig_{tag}", name="sig")
                nc.scalar.activation(sig[:], L[:], AF.Exp, bias=0.0, scale=0.5)
                return rstd, sig

            # ================= per-sample state =================
            state = [dict() for _ in range(B_LOC)]

            def warmup(n=56):
                """Start the HAM activity window while the x DMA lands; the
                first projection matmuls continue it.  fp16 identity: the f32
                version cost 333ns LDW + 400ns MM *each* and blocked the PE
                queue for 13.5us."""
                junk = banks.tile([128, 2, 512], F32, tag="bank")
                for _ in range(n):
                    nc.tensor.matmul(
                        junk[:, 0, 0:64], identh[:], identh[:, 0:64],
                        start=True, stop=True,
                    )

            def load_x(s, fine=True):
                """One tile per 1024-col pair: the first projection matmuls
                then wait only on their own pair's two DMA chunks, not on
                all 8 (the sync engine needs ~650ns to *issue* each DMA, so
                whole-tile deps cost ~4us at kernel start)."""
                tiles = []
                for pr in range(NPAIR):
                    xt = xpool.tile([128, KCH, 1024], F16, tag="x", name="xt")
                    cols = slice(1024 * pr, 1024 * (pr + 1))
                    for k in range(KCH):
                        nc.sync.dma_start(
                            xt[:, k, :],
                            x_ext[s, 128 * k:128 * (k + 1), cols],
                        )
                    tiles.append(xt)
                state[s]["x"] = tiles

            def proj_pair(s, pr, defer=False):
                """One 1024-col pair of all 3 projections: 12 matmuls, raw
                cast-evict to fp16, stats on SBUF.  defer=True stashes the
                evicts+stats for flush_pp() -- the deferred pair's matmuls
                fill the PE during the other sample's softmax without its
                D/A work queueing ahead of the softmax chain."""
                x_sb = state[s]["x"][pr]
                if pr == 0:
                    for p in range(3):
                        pool_p = grawp if p == 2 else rawp
                        state[s][f"praw{p}"] = pool_p.tile(
                            [128, N], F16,
                            tag="graw" if p == 2 else "praw", name=f"praw{p}")
                        state[s][f"stats{p}"] = small.tile(
                            [128, 2 * NPAIR, 6], F32, tag=f"stats{p}",
                            name=f"stats{p}")
                ps3 = [banks.tile([128, 2, 512], F32, tag="bank", name=f"pp{p}")
                       for p in range(3)]
                for p in range(3):
                    for h in range(2):
                        cols = slice(512 * h, 512 * (h + 1))
                        for k in range(KCH):
                            nc.tensor.matmul(
                                ps3[p][:, h, :], w_sb[:, k, p, :], x_sb[:, k, cols],
                                start=(k == 0), stop=(k == KCH - 1),
                            )
                if defer:
                    state[s]["dpp"] = (pr, ps3)
                    return
                _pp_tail(s, pr, ps3)

            def flush_pp(s):
                pr, ps3 = state[s].pop("dpp")
                _pp_tail(s, pr, ps3)

            def _pp_tail(s, pr, ps3):
                for p in range(3):
                    praw = state[s][f"praw{p}"]
                    c0 = 1024 * pr
                    # pinned to ACT: DVE must stay stats-only here, or the
                    # theta/phi aggregations queue behind evicts and stall
                    # the transposes (the psum ring only needs the evict)
                    evict(praw[:, c0:c0 + 1024], ps3[p][:], EV_PROJ, eng="A")
                    stats = state[s][f"stats{p}"]
                    for h in range(2):
                        nc.vector.bn_stats(
                            stats[:, 2 * pr + h, :],
                            praw[:, c0 + 512 * h:c0 + 512 * (h + 1)],
                        )
                        load["D"] += 577.0

            def proj_finish(s, p):
                """Aggregate stats; out-of-place apply.
                theta: relu(x-mean) -> pairbuf, rstd into softmax scale.
                phi:   (x-mean)*rstd -> pairbuf, relu in transpose-evict.
                g:     relu(x-mean) fp16 -> gtil, rstd into attnT."""
                mv = small.tile([128, 2], F32, tag=f"mv{p}", name="mv")
                nc.vector.bn_aggr(mv[:], state[s][f"stats{p}"][:])
                load["D"] += 300.0
                rstd, _ = rstd_chain(mv, tag=f"p{p}")
                negb = small.tile([128, 1], F32, tag=f"negb{p}", name="negb")
                if p == 1:
                    nc.vector.tensor_scalar(
                        negb[:], mv[:, 0:1], rstd[:], -1.0,
                        op0=AL.mult, op1=AL.mult,
                    )
                else:
                    nc.vector.tensor_scalar_mul(negb[:], mv[:, 0:1], -1.0)
                load["D"] += 150.0
                praw = state[s][f"praw{p}"]
                if p == 2:
                    gt = gtp.tile([128, N], F16, tag="gt", name="gt")
                    state[s]["gtil"] = gt
                else:
                    state[s][f"pb{p}"] = []
                for pr in range(NPAIR):
                    cols = slice(1024 * pr, 1024 * (pr + 1))
                    if p == 2:
                        dst = gt[:, cols]
                    else:
                        pb = pbp.tile([128, 1024], F16, tag="pb", name="pb")
                        state[s][f"pb{p}"].append(pb)
                        dst = pb[:]
                    eng = "D"
                    load["D"] += APPLY_1024["D"]
                    if eng == "A":
                        if p == 1:
                            nc.scalar.activation(
                                dst, praw[:, cols], AF.Identity,
                                bias=negb[:], scale=rstd[:],
                            )
                        else:
                            nc.scalar.activation(
                                dst, praw[:, cols], AF.Relu, bias=negb[:],
                            )
                    else:
                        if p == 1:
                            nc.vector.tensor_scalar(
                                dst, praw[:, cols], mv[:, 0:1], rstd[:],
                                op0=AL.subtract, op1=AL.mult,
                            )
                        else:
                            nc.vector.tensor_scalar(
                                dst, praw[:, cols], mv[:, 0:1], 0.0,
                                op0=AL.subtract, op1=AL.max,
                            )
                state[s]["tpg"[p] + "rstd"] = rstd

            def tn(s, p):
                """PE transposes of theta~ (p=0, slot 1) / phi-scaled (p=1,
                slot 0, relu on evict) into the shared fp16 TP buffer."""
                pbs = state[s][f"pb{p}"]
                slot = 1 - p
                if p == 0:
                    TP = tpp.tile([128, NT, 2, 128], F16, tag="TP", name="TP")
                    state[s]["TP"] = TP
                TP = state[s]["TP"]
                for grp in range(4):
                    pb = pbs[grp]
                    tp = banks.tile([128, 8, 128], F16, tag="bank", name="tps")
                    for j in range(8):
                        nc.tensor.transpose(
                            tp[:, j, :], pb[:, 128 * j:128 * (j + 1)], identh[:],
                        )
                    dst = TP[:, 8 * grp:8 * (grp + 1), slot, :]
                    evict(dst, tp[:], EV_TP, func="relu" if p == 1 else None)

            def gram(s):
                TP = state[s]["TP"]
                f_ps = psmall.tile([128, 256], F32, tag="fg", name="fg")
                for t in range(NT):
                    nc.tensor.matmul(
                        f_ps[:], TP[:, t, 1, :],
                        TP[:, t, :, :].rearrange("p a b -> p (a b)"),
                        start=(t == 0), stop=(t == NT - 1),
                    )
                state[s]["f_ps"] = f_ps

            def softmax(s):
                f_ap = state[s]["f_ps"][:, 0:128]
                trstd = state[s]["trstd"]
                negmaxr = small.tile([128, 1], F32, tag="negmaxr", name="negmaxr")
                nc.vector.tensor_reduce(
                    negmaxr[:], f_ap, axis=mybir.AxisListType.X,
                    op=AL.max, negate=True,
                )
                load["D"] += 300.0
                negmax = small.tile([128, 1], F32, tag="negmax", name="negmax")
                nc.vector.tensor_scalar_mul(negmax[:], negmaxr[:], trstd[:])
                attn_e = small.tile([128, 128], F32, tag="attn_e", name="attn_e")
                sumexp = small.tile([128, 1], F32, tag="sumexp", name="sumexp")
                nc.scalar.activation(
                    attn_e[:], f_ap, AF.Exp,
                    bias=negmax[:], scale=trstd[:], accum_out=sumexp[:],
                )
                rsum = small.tile([128, 1], F32, tag="rsum", name="rsum")
                nc.vector.reciprocal(rsum[:], sumexp[:])
                attn_n = small.tile([128, 128], F16, tag="attn_n", name="attn_n")
                nc.vector.tensor_scalar_mul(attn_n[:], attn_e[:], rsum[:])
                load["D"] += 350.0
                state[s]["attn_n"] = attn_n

            def attn_transpose(s):
                at_ps = psmall.tile([128, 128], F16, tag="at", name="at")
                nc.tensor.transpose(at_ps[:], state[s]["attn_n"][:], identh[:])
                attnT = small.tile([128, 128], F16, tag="attnT", name="attnT")
                nc.vector.tensor_scalar_mul(attnT[:], at_ps[:], state[s]["grstd"][:])
                load["D"] += 250.0
                state[s]["attnT"] = attnT

            def zstage(s):
                gtil = state[s]["gtil"]
                attnT = state[s]["attnT"]
                Z = zp.tile([128, N], F16, tag="z", name="Z")
                g3 = gtil[:].rearrange("p (c q) -> p c q", q=NT)
                for grp in range(4):
                    ps = banks.tile([128, 8, 128], F32, tag="bank", name="zps")
                    for j in range(8):
                        q = 8 * grp + j
                        nc.tensor.matmul(
                            ps[:, j, :], g3[:, :, q], attnT[:],
                            start=True, stop=True,
                        )
                    c0 = 1024 * grp
                    evict(Z[:, c0:c0 + 1024], ps[:], EV_Z)
                state[s]["Z"] = Z

            def conv_stats(s, oc, defer=False):
                """First conv pass: matmul then bn_stats straight on psum
                (no hold needed -- the output pass recomputes).  defer=True
                emits only the matmuls (+pr0 stats, keeping the psum-ring
                WAR pointing at an already-emitted reader) so they can fill
                the PE during a softmax window; flush_cs() emits the rest."""
                Z = state[s]["Z"]
                stats2 = small.tile([128, 2 * NPAIR, 6], F32, tag="stats2",
                                    name="stats2")
                held = []
                for pr in range(NPAIR):
                    ps = banks.tile([128, 2, 512], F32, tag="bank", name="cps")
                    for h in range(2):
                        cols = slice(1024 * pr + 512 * h, 1024 * pr + 512 * (h + 1))
                        nc.tensor.matmul(
                            ps[:, h, :], ww_sb[:, 128 * oc:128 * (oc + 1)],
                            Z[:, cols], start=True, stop=True,
                        )
                    # deferring ALL pairs' stats is safe: the pr3 matmul's
                    # ring WAR then waits on a later-emitted DVE stat, which
                    # is cross-engine and cycle-free (the same-engine case is
                    # the deadlocking one)
                    if defer:
                        held.append((pr, ps))
                    else:
                        for h in range(2):
                            nc.vector.bn_stats(stats2[:, 2 * pr + h, :], ps[:, h, :])
                            load["D"] += 660.0
                if defer:
                    state[s]["dcs"] = (oc, stats2, held)
                    return
                _cs_tail(s, oc, stats2)

            def flush_cs(s):
                oc, stats2, held = state[s].pop("dcs")
                for pr, ps in held:
                    for h in range(2):
                        nc.vector.bn_stats(stats2[:, 2 * pr + h, :], ps[:, h, :])
                        load["D"] += 660.0
                _cs_tail(s, oc, stats2)

            def _cs_tail(s, oc, stats2):
                mv2 = small.tile([128, 2], F32, tag="mv2", name="mv2")
                nc.vector.bn_aggr(mv2[:], stats2[:])
                load["D"] += 300.0
                rstd2, sig2 = rstd_chain(mv2, tag=f"c{oc}", want_sigma=True)
                negmr2 = small.tile([128, 1], F32, tag=f"negmr2_{oc}", name="negmr2")
                nc.vector.tensor_scalar(
                    negmr2[:], mv2[:, 0:1], rstd2[:], -1.0,
                    op0=AL.mult, op1=AL.mult,
                )
                diag2 = small.tile([128, 128], F16, tag=f"diag2_{oc}", name="diag2")
                nc.vector.tensor_scalar_mul(diag2[:], ident32[:], sig2[:])
                load["D"] += 400.0
                state[s].setdefault("cstats", {})[oc] = (rstd2, negmr2, diag2)

            def conv_out(s, oc, prs, alt=False):
                """Recompute conv chunks + diag residual, evict normalized f32,
                DMA out."""
                Z = state[s]["Z"]
                rstd2, negmr2, diag2 = state[s]["cstats"][oc]
                for pr in prs:
                    x_sb = state[s]["x"][pr]
                    ps = banks.tile([128, 2, 512], F32, tag="bank", name="ops")
                    for h in range(2):
                        cols = slice(1024 * pr + 512 * h, 1024 * pr + 512 * (h + 1))
                        xcols = slice(512 * h, 512 * (h + 1))
                        nc.tensor.matmul(
                            ps[:, h, :], ww_sb[:, 128 * oc:128 * (oc + 1)],
                            Z[:, cols], start=True, stop=False,
                        )
                        nc.tensor.matmul(
                            ps[:, h, :], diag2[:], x_sb[:, oc, xcols],
                            start=False, stop=True,
                        )
                    oring = orp.tile([128, 2, 512], F16, tag="oring", name="oring")
                    evict(oring[:], ps[:], EV_TS,
                          func="ts", bias=negmr2[:], scale=rstd2[:],
                          eng=("D" if pr % 2 else "A") if alt else None)
                    cols = slice(1024 * pr, 1024 * (pr + 1))
                    nc.sync.dma_start(
                        out_ext[s, 128 * oc:128 * (oc + 1), cols], oring[:],
                    )

            # ================= pipelined emission =================
            # PE queue stays dense: sample 1's proj matmuls run during
            # sample 0's finish/attn windows so HAM never sees an idle
            # MID window until the very tail.
            load_x(0)
            warmup()
            load_x(1)
            for pr in range(NPAIR):
                proj_pair(0, pr)
            proj_finish(0, 0)
            proj_finish(0, 1)
            proj_pair(1, 0)
            proj_finish(0, 2)
            proj_pair(1, 1)
            tn(0, 0)
            proj_pair(1, 2)
            tn(0, 1)
            gram(0)
            proj_pair(1, 3, defer=True)
            softmax(0)
            attn_transpose(0)
            flush_pp(1)
            zstage(0)
            proj_finish(1, 0)
            proj_finish(1, 1)
            proj_finish(1, 2)
            tn(1, 0)
            tn(1, 1)
            gram(1)
            conv_stats(0, 0, defer=True)
            softmax(1)
            attn_transpose(1)
            flush_cs(0)
            zstage(1)
            conv_stats(0, 1)
            conv_out(0, 0, range(NPAIR))
            conv_stats(1, 0)
            conv_out(0, 1, range(NPAIR))
            conv_stats(1, 1)
            conv_out(1, 0, range(NPAIR), alt=True)
            conv_out(1, 1, range(NPAIR), alt=True)

    nc.compile()
    return nc


def _get_nc():
    if "nc" not in _CACHE:
        _CACHE["nc"] = build_nc()
    return _CACHE["nc"]


def _prep_in_maps(x, g_w, theta_w, phi_w, W_w):
    # stacked projection lhsT: [c, ci] chunks -> [128, KCH, 3, 128]
    w3 = np.stack(
        [theta_w.T.reshape(KCH, 128, CI), phi_w.T.reshape(KCH, 128, CI),
         g_w.T.reshape(KCH, 128, CI)],
        axis=2,
    )  # [KCH, 128, 3, 128]
    w3 = np.ascontiguousarray(w3.transpose(1, 0, 2, 3)).astype(np.float16)
    ww = np.ascontiguousarray(W_w.T).astype(np.float16)  # [CI, C]
    xr = np.ascontiguousarray(x.reshape(B, C, N)).astype(np.float16)
    in_maps = []
    for c in range(N_CORES):
        in_maps.append({
            "x": xr[B_LOC * c:B_LOC * (c + 1)],
            "w3": w3,
            "ww": ww,
        })
    return in_maps


def kernel(x, g_w, g_b, theta_w, theta_b, phi_w, phi_b, W_w, W_b, **_ignored):
    # biases are mathematically dropped by the InstanceNorms
    nc = _get_nc()
    in_maps = _prep_in_maps(x, g_w, theta_w, phi_w, W_w)
    res = run_bass_kernel_spmd(nc, in_maps, core_ids=list(range(N_CORES)))
    outs = [res.results[c]["out"].reshape(B_LOC, C, H, W) for c in range(N_CORES)]
    return np.concatenate(outs, axis=0).astype(np.float32)


def _install_ntff_hook():
    """Provide antenv.axon_hooks if the image lacks it (see trn_boot.py)."""
    import types
    try:
        from antenv.axon_hooks import get_axon_ntff_profile_hook  # noqa: F401
        return
    except ImportError:
        pass
    import contextlib
    import ctypes

    so_path = "/opt/axon/libaxon_pjrt.so"
    lib = ctypes.CDLL(so_path)
    if not hasattr(lib, "axon_start_nrt_profile"):
        hook = None
    else:
        lib.axon_start_nrt_profile.argtypes = [
            ctypes.POINTER(ctypes.c_int64), ctypes.c_size_t]
        lib.axon_start_nrt_profile.restype = ctypes.c_int64
        lib.axon_stop_nrt_profile.argtypes = [ctypes.c_char_p]
        lib.axon_stop_nrt_profile.restype = ctypes.c_int64

        @contextlib.contextmanager
        def hook(output_dir, device_ids):
            import jax
            jax.devices()
            if device_ids:
                ids = (ctypes.c_int64 * len(device_ids))(*device_ids)
                rc = lib.axon_start_nrt_profile(ids, len(device_ids))
            else:
                rc = lib.axon_start_nrt_profile(None, 0)
            if rc != 0:
                raise RuntimeError(f"axon_start_nrt_profile rc={rc}")
            try:
                yield
            finally:
                n = lib.axon_stop_nrt_profile(str(output_dir).encode())
                if n <= 0:
                    raise RuntimeError(f"axon_stop_nrt_profile rc={n}")

    mod = types.ModuleType("antenv.axon_hooks")
    mod.get_axon_ntff_profile_hook = lambda: hook
    mod.set_axon_ntff_profile_hook = lambda h: None
    sys.modules["antenv.axon_hooks"] = mod


def run_traced(x, g_w, g_b, theta_w, theta_b, phi_w, phi_b, W_w, W_b, **_ignored):
    """Like kernel() but with NTFF profiling; returns (out, BassKernelResults)."""
    _install_ntff_hook()
    nc = _get_nc()
    in_maps = _prep_in_maps(x, g_w, theta_w, phi_w, W_w)
    res = run_bass_kernel_spmd(
        nc, in_maps, core_ids=list(range(N_CORES)), trace=True
    )
    outs = [res.results[c]["out"].reshape(B_LOC, C, H, W) for c in range(N_CORES)]
    return np.concatenate(outs, axis=0).astype(np.float32), res


# revision 44
# speedup vs baseline: 1.0060x; 1.0060x over previous
"""ChannelAwareAttentionModule TRN2 kernel (v12: cross-sample PE interleave).

Math (per sample s; biases are no-ops because InstanceNorm removes them):
  thetaN/phiN/gN = relu(instnorm(w @ x))        [Ci=128, N=4096]
  f = thetaN @ phiN^T; attn = softmax(f, axis=1)
  y = attn @ gN; Z[ci, q*128+r] = y[r, 32*ci+q]
  out = instnorm(W_w @ Z) + x                   [256, 4096]

Precision: fp16 (10 mantissa bits) for x/w/projections/gram inputs -- the
gram itself accumulates fp32 in PSUM so logits (~650 +- 70) stay accurate.

Factorizations (relu(rstd*(x-mean)) = rstd*relu(x-mean), rstd>0):
  theta: apply = (x sub mean) max 0; rstd folds into the softmax Exp scale.
  phi:   apply = (x sub mean) mult rstd; relu moves into the transpose-evict.
  g:     fp16 apply; rstd folds into attn^T rows.
  conv:  recompute + diag(sigma) @ x accumulated in psum makes the final
         evict a 2-op scale+bias that lands instnorm(U) + x exactly.
  rstd = exp(-0.5*ln(var+eps)); the act-table patch keeps ln/exp/relu/
  identity/copy in ONE table (natural_log_exp_and_others) -- no reloads.

Scheduling: the PE HAM clock-gate drops to 1.2 GHz after any ~3.4us
window of low PE activity, and costs ~40us/run when the proj_finish /
softmax windows idle the PE.  Sample 1's projection matmuls are emitted
interleaved with sample 0's finish/attn phases (praw/stat rings sized so
the cross-sample WAR waits never point at later-emitted readers, which
would deadlock the in-order queues).  Projection evicts are pinned to
ACT so DVE stays stats-only (aggregations aren't queued behind evicts);
applies are pinned to DVE (530ns vs 1140ns on ACT, and same-engine as
the aggregation they depend on).  The conv tail pipelines
stats(s,oc)/out(s,oc) so PE recompute overlaps the next stats pass.

Sharding: data-parallel over batch, 2 samples per core, 8 cores.
"""
import sys

sys.path.insert(0, "/opt/trn_rl_repo")

import numpy as np

import concourse.bass as bass
import concourse.bacc as bacc
import concourse.tile as tile
from concourse import mybir
from concourse.bass_utils import run_bass_kernel_spmd
from concourse.masks import make_identity

N_CORES = 8
B, C, CI, H, W = 16, 256, 128, 64, 64
N = H * W  # 4096
B_LOC = B // N_CORES  # 2 samples per core
KCH = C // 128  # 2 contraction chunks of input channels
NT = N // 128  # 32 column tiles
NPAIR = 4  # 4 psum pairs of 1024 f32 per 4096 cols
EPS = 1e-5

F32 = mybir.dt.float32
F16 = mybir.dt.float16

_CACHE = {}


def _patch_act_tables():
    """Steer the ACT-table pass to the one table that serves every function
    this kernel uses (ln, exp, relu, identity, copy, square).

    Without this the pass greedily alternates between `natural_log` (has ln,
    no exp) and `exp_and_others` (has exp, no ln), reloading a 1.3us table
    ~17 times per kernel.  We strip our functions from every other table so
    `natural_log_exp_and_others` is the only candidate; the dict keeps its
    original order/length so act_func_set_id indices stay valid."""
    import concourse.hw_specs as hw_specs

    if getattr(hw_specs.get_activation_tables, "_caam_patched", False):
        return
    real_fn = hw_specs.get_activation_tables
    AF = mybir.ActivationFunctionType
    ours = {AF.Ln, AF.Exp, AF.Identity, AF.Relu, AF.Copy, AF.Square}
    keep = "natural_log_exp_and_others"

    def patched(arch, _real=real_fn, _ours=ours, _keep=keep):
        tables = _real(arch)
        if _keep not in tables:
            return tables
        return {
            name: (set(fns) if name == _keep else set(fns) - _ours)
            for name, fns in tables.items()
        }

    patched._caam_patched = True
    hw_specs.get_activation_tables = patched
    bacc.get_activation_tables = patched
    try:
        import concourse.bass_interp as bass_interp

        bass_interp.get_activation_tables = patched
    except ImportError:
        pass


def build_nc():
    _patch_act_tables()
    nc = bacc.Bacc("TRN2", target_bir_lowering=False)

    x_ext = nc.declare_dram_parameter("x", [B_LOC, C, N], F16, isOutput=False)
    # stacked projection weights, host layout [128, KCH, 3, 128] = [c128, k, proj, ci]
    w_ext = nc.declare_dram_parameter("w3", [128, KCH, 3, CI], F16, isOutput=False)
    ww_ext = nc.declare_dram_parameter("ww", [CI, C], F16, isOutput=False)
    # fp16 output halves the HBM write traffic; the host casts back to f32
    # (instnorm+residual values are O(1), so fp16 adds only ~2e-4 error)
    out_ext = nc.declare_dram_parameter("out", [B_LOC, C, N], F16, isOutput=True)

    AL = mybir.AluOpType
    AF = mybir.ActivationFunctionType

    with tile.TileContext(nc) as tc:
        from contextlib import ExitStack

        with ExitStack() as ctx:
            consts = ctx.enter_context(tc.tile_pool(name="consts", bufs=1))
            xpool = ctx.enter_context(tc.tile_pool(name="xpool", bufs=8))
            rawp = ctx.enter_context(tc.tile_pool(name="rawp", bufs=4))
            grawp = ctx.enter_context(tc.tile_pool(name="grawp", bufs=2))
            pbp = ctx.enter_context(tc.tile_pool(name="pbp", bufs=8))
            gtp = ctx.enter_context(tc.tile_pool(name="gtp", bufs=2))
            tpp = ctx.enter_context(tc.tile_pool(name="tpp", bufs=1))
            zp = ctx.enter_context(tc.tile_pool(name="zp", bufs=2))
            orp = ctx.enter_context(tc.tile_pool(name="orp", bufs=4))
            small = ctx.enter_context(tc.tile_pool(name="small", bufs=2))
            banks = ctx.enter_context(tc.tile_pool(name="banks", bufs=3, space="PSUM"))
            psmall = ctx.enter_context(tc.tile_pool(name="psmall", bufs=1, space="PSUM"))

            # ---- constants ----
            ident32 = consts.tile([128, 128], F32)
            make_identity(nc, ident32[:])
            identh = consts.tile([128, 128], F16)
            nc.vector.tensor_copy(identh[:], ident32[:])
            w_sb = consts.tile([128, KCH, 3, CI], F16)
            nc.scalar.dma_start(w_sb[:], w_ext[:])
            ww_sb = consts.tile([CI, C], F16)
            nc.scalar.dma_start(ww_sb[:], ww_ext[:])
            eps_t = consts.tile([128, 1], F32)
            nc.vector.memset(eps_t[:], EPS)

            # ---- greedy engine-balance ----------------------------------
            # Pool has no PSUM port: PSUM work goes to DVE ("D") or ACT ("A").
            load = {"D": 0.0, "A": 0.0}
            ENG = {"D": nc.vector, "A": nc.scalar}

            def pick(costs):
                k = min(costs, key=lambda e: load[e] + costs[e])
                load[k] += costs[k]
                return k

            # per-op costs (ns) for 1024 cols, fit to ntff measurements
            EV_PROJ = {"D": 1170.0, "A": 1160.0}  # psum f32 -> sbuf f16 cast
            EV_TP = {"D": 660.0, "A": 570.0}      # psum f16 -> sbuf f16 (2x)
            EV_Z = {"D": 1170.0, "A": 1160.0}     # psum f32 -> sbuf f16 cast
            EV_TS = {"D": 1300.0, "A": 1240.0}    # psum f32 scale+bias -> f32
            APPLY_1024 = {"D": 530.0, "A": 1140.0}  # sbuf f16 2-op

            def evict(dst, src, costs, func=None, bias=None, scale=None,
                      eng=None):
                if eng is None:
                    eng = pick(costs)
                else:
                    load[eng] += costs[eng]
                if eng == "A":
                    if func == "relu":
                        nc.scalar.activation(dst, src, AF.Relu)
                    elif func == "ts":
                        nc.scalar.activation(
                            dst, src, AF.Identity, bias=bias, scale=scale)
                    else:
                        nc.scalar.copy(dst, src)
                else:
                    if func == "relu":
                        nc.vector.tensor_scalar_max(dst, src, 0.0)
                    elif func == "ts":
                        nc.vector.tensor_scalar(
                            dst, src, scale, bias, op0=AL.mult, op1=AL.add)
                    else:
                        nc.vector.tensor_copy(dst, src)

            def rstd_chain(mv, tag, want_sigma=False):
                """mv [128,2]=(mean,var) -> rstd=exp(-ln(var+eps)/2) (+sigma)."""
                L = small.tile([128, 1], F32, tag="lnv", name="lnv")
                nc.scalar.activation(L[:], mv[:, 1:2], AF.Ln, bias=eps_t[:], scale=1.0)
                rstd = small.tile([128, 1], F32, tag=f"rstd_{tag}", name="rstd")
                nc.scalar.activation(rstd[:], L[:], AF.Exp, bias=0.0, scale=-0.5)
                if not want_sigma:
                    return rstd, None
                sig = small.tile([128, 1], F32, tag=f"sig_{tag}", name="sig")
                nc.scalar.activation(sig[:], L[:], AF.Exp, bias=0.0, scale=0.5)
                return rstd, sig

            # ================= per-sample state =================
            state = [dict() for _ in range(B_LOC)]

            def warmup(n=56):
                """Start the HAM activity window while the x DMA lands; the
                first projection matmuls continue it.  fp16 identity: the f32
                version cost 333ns LDW + 400ns MM *each* and blocked the PE
                queue for 13.5us."""
                junk = banks.tile([128, 2, 512], F32, tag="bank")
                for _ in range(n):
                    nc.tensor.matmul(
                        junk[:, 0, 0:64], identh[:], identh[:, 0:64],
                        start=True, stop=True,
                    )

            def load_x(s, fine=True):
                """One tile per 1024-col pair: the first projection matmuls
                then wait only on their own pair's two DMA chunks, not on
                all 8 (the sync engine needs ~650ns to *issue* each DMA, so
                whole-tile deps cost ~4us at kernel start)."""
                tiles = []
                for pr in range(NPAIR):
                    xt = xpool.tile([128, KCH, 1024], F16, tag="x", name="xt")
                    cols = slice(1024 * pr, 1024 * (pr + 1))
                    for k in range(KCH):
                        nc.sync.dma_start(
                            xt[:, k, :],
                            x_ext[s, 128 * k:128 * (k + 1), cols],
                        )
                    tiles.append(xt)
                state[s]["x"] = tiles

            def proj_pair(s, pr, defer=False):
                """One 1024-col pair of all 3 projections: 12 matmuls, raw
                cast-evict to fp16, stats on SBUF.  defer=True stashes the
                evicts+stats for flush_pp() -- the deferred pair's matmuls
                fill the PE during the other sample's softmax without its
                D/A work queueing ahead of the softmax chain."""
                x_sb = state[s]["x"][pr]
                if pr == 0:
                    for p in range(3):
                        pool_p = grawp if p == 2 else rawp
                        state[s][f"praw{p}"] = pool_p.tile(
                            [128, N], F16,
                            tag="graw" if p == 2 else "praw", name=f"praw{p}")
                        state[s][f"stats{p}"] = small.tile(
                            [128, 2 * NPAIR, 6], F32, tag=f"stats{p}",
                            name=f"stats{p}")
                ps3 = [banks.tile([128, 2, 512], F32, tag="bank", name=f"pp{p}")
                       for p in range(3)]
                for p in range(3):
                    for h in range(2):
                        cols = slice(512 * h, 512 * (h + 1))
                        for k in range(KCH):
                            nc.tensor.matmul(
                                ps3[p][:, h, :], w_sb[:, k, p, :], x_sb[:, k, cols],
                                start=(k == 0), stop=(k == KCH - 1),
                            )
                if defer:
                    state[s]["dpp"] = (pr, ps3)
                    return
                _pp_tail(s, pr, ps3)

            def flush_pp(s):
                pr, ps3 = state[s].pop("dpp")
                _pp_tail(s, pr, ps3)

            def _pp_tail(s, pr, ps3):
                for p in range(3):
                    praw = state[s][f"praw{p}"]
                    c0 = 1024 * pr
                    # pinned to ACT: DVE must stay stats-only here, or the
                    # theta/phi aggregations queue behind evicts and stall
                    # the transposes (the psum ring only needs the evict)
                    evict(praw[:, c0:c0 + 1024], ps3[p][:], EV_PROJ, eng="A")
                    stats = state[s][f"stats{p}"]
                    for h in range(2):
                        nc.vector.bn_stats(
                            stats[:, 2 * pr + h, :],
                            praw[:, c0 + 512 * h:c0 + 512 * (h + 1)],
                        )
                        load["D"] += 577.0

            def proj_finish(s, p):
                """Aggregate stats; out-of-place apply.
                theta: relu(x-mean) -> pairbuf, rstd into softmax scale.
                phi:   (x-mean)*rstd -> pairbuf, relu in transpose-evict.
                g:     relu(x-mean) fp16 -> gtil, rstd into attnT."""
                mv = small.tile([128, 2], F32, tag=f"mv{p}", name="mv")
                nc.vector.bn_aggr(mv[:], state[s][f"stats{p}"][:])
                load["D"] += 300.0
                rstd, _ = rstd_chain(mv, tag=f"p{p}")
                negb = small.tile([128, 1], F32, tag=f"negb{p}", name="negb")
                if p == 1:
                    nc.vector.tensor_scalar(
                        negb[:], mv[:, 0:1], rstd[:], -1.0,
                        op0=AL.mult, op1=AL.mult,
                    )
                else:
                    nc.vector.tensor_scalar_mul(negb[:], mv[:, 0:1], -1.0)
                load["D"] += 150.0
                praw = state[s][f"praw{p}"]
                if p == 2:
                    gt = gtp.tile([128, N], F16, tag="gt", name="gt")
                    state[s]["gtil"] = gt
                else:
                    state[s][f"pb{p}"] = []
                for pr in range(NPAIR):
                    cols = slice(1024 * pr, 1024 * (pr + 1))
                    if p == 2:
                        dst = gt[:, cols]
                    else:
                        pb = pbp.tile([128, 1024], F16, tag="pb", name="pb")
                        state[s][f"pb{p}"].append(pb)
                        dst = pb[:]
                    eng = "D"
                    load["D"] += APPLY_1024["D"]
                    if eng == "A":
                        if p == 1:
                            nc.scalar.activation(
                                dst, praw[:, cols], AF.Identity,
                                bias=negb[:], scale=rstd[:],
                            )
                        else:
                            nc.scalar.activation(
                                dst, praw[:, cols], AF.Relu, bias=negb[:],
                            )
                    else:
                        if p == 1:
                            nc.vector.tensor_scalar(
                                dst, praw[:, cols], mv[:, 0:1], rstd[:],
                                op0=AL.subtract, op1=AL.mult,
                            )
                        else:
                            nc.vector.tensor_scalar(
                                dst, praw[:, cols], mv[:, 0:1], 0.0,
                                op0=AL.subtract, op1=AL.max,
                            )
                state[s]["tpg"[p] + "rstd"] = rstd

            def tn(s, p):
                """PE transposes of theta~ (p=0, slot 1) / phi-scaled (p=1,
                slot 0, relu on evict) into the shared fp16 TP buffer."""
                pbs = state[s][f"pb{p}"]
                slot = 1 - p
                if p == 0:
                    TP = tpp.tile([128, NT, 2, 128], F16, tag="TP", name="TP")
                    state[s]["TP"] = TP
                TP = state[s]["TP"]
                for grp in range(4):
                    pb = pbs[grp]
                    tp = banks.tile([128, 8, 128], F16, tag="bank", name="tps")
                    for j in range(8):
                        nc.tensor.transpose(
                            tp[:, j, :], pb[:, 128 * j:128 * (j + 1)], identh[:],
                        )
                    dst = TP[:, 8 * grp:8 * (grp + 1), slot, :]
                    evict(dst, tp[:], EV_TP, func="relu" if p == 1 else None)

            def gram(s):
                TP = state[s]["TP"]
                f_ps = psmall.tile([128, 256], F32, tag="fg", name="fg")
                for t in range(NT):
                    nc.tensor.matmul(
                        f_ps[:], TP[:, t, 1, :],
                        TP[:, t, :, :].rearrange("p a b -> p (a b)"),
                        start=(t == 0), stop=(t == NT - 1),
                    )
                state[s]["f_ps"] = f_ps

            def softmax(s):
                f_ap = state[s]["f_ps"][:, 0:128]
                trstd = state[s]["trstd"]
                negmaxr = small.tile([128, 1], F32, tag="negmaxr", name="negmaxr")
                nc.vector.tensor_reduce(
                    negmaxr[:], f_ap, axis=mybir.AxisListType.X,
                    op=AL.max, negate=True,
                )
                load["D"] += 300.0
                negmax = small.tile([128, 1], F32, tag="negmax", name="negmax")
                nc.vector.tensor_scalar_mul(negmax[:], negmaxr[:], trstd[:])
                attn_e = small.tile([128, 128], F32, tag="attn_e", name="attn_e")
                sumexp = small.tile([128, 1], F32, tag="sumexp", name="sumexp")
                nc.scalar.activation(
                    attn_e[:], f_ap, AF.Exp,
                    bias=negmax[:], scale=trstd[:], accum_out=sumexp[:],
                )
                rsum = small.tile([128, 1], F32, tag="rsum", name="rsum")
                nc.vector.reciprocal(rsum[:], sumexp[:])
                attn_n = small.tile([128, 128], F16, tag="attn_n", name="attn_n")
                nc.vector.tensor_scalar_mul(attn_n[:], attn_e[:], rsum[:])
                load["D"] += 350.0
                state[s]["attn_n"] = attn_n

            def attn_transpose(s):
                at_ps = psmall.tile([128, 128], F16, tag="at", name="at")
                nc.tensor.transpose(at_ps[:], state[s]["attn_n"][:], identh[:])
                attnT = small.tile([128, 128], F16, tag="attnT", name="attnT")
                nc.vector.tensor_scalar_mul(attnT[:], at_ps[:], state[s]["grstd"][:])
                load["D"] += 250.0
                state[s]["attnT"] = attnT

            def zstage(s):
                gtil = state[s]["gtil"]
                attnT = state[s]["attnT"]
                Z = zp.tile([128, N], F16, tag="z", name="Z")
                g3 = gtil[:].rearrange("p (c q) -> p c q", q=NT)
                for grp in range(4):
                    ps = banks.tile([128, 8, 128], F32, tag="bank", name="zps")
                    for j in range(8):
                        q = 8 * grp + j
                        nc.tensor.matmul(
                            ps[:, j, :], g3[:, :, q], attnT[:],
                            start=True, stop=True,
                        )
                    c0 = 1024 * grp
                    evict(Z[:, c0:c0 + 1024], ps[:], EV_Z)
                state[s]["Z"] = Z

            def conv_stats(s, oc, defer=False):
                """First conv pass: matmul then bn_stats straight on psum
                (no hold needed -- the output pass recomputes).  defer=True
                emits only the matmuls (+pr0 stats, keeping the psum-ring
                WAR pointing at an already-emitted reader) so they can fill
                the PE during a softmax window; flush_cs() emits the rest."""
                Z = state[s]["Z"]
                stats2 = small.tile([128, 2 * NPAIR, 6], F32, tag="stats2",
                                    name="stats2")
                held = []
                for pr in range(NPAIR):
                    ps = banks.tile([128, 2, 512], F32, tag="bank", name="cps")
                    for h in range(2):
                        cols = slice(1024 * pr + 512 * h, 1024 * pr + 512 * (h + 1))
                        nc.tensor.matmul(
                            ps[:, h, :], ww_sb[:, 128 * oc:128 * (oc + 1)],
                            Z[:, cols], start=True, stop=True,
                        )
                    # deferring ALL pairs' stats is safe: the pr3 matmul's
                    # ring WAR then waits on a later-emitted DVE stat, which
                    # is cross-engine and cycle-free (the same-engine case is
                    # the deadlocking one)
                    if defer:
                        held.append((pr, ps))
                    else:
                        for h in range(2):
                            nc.vector.bn_stats(stats2[:, 2 * pr + h, :], ps[:, h, :])
                            load["D"] += 660.0
                if defer:
                    state[s]["dcs"] = (oc, stats2, held)
                    return
                _cs_tail(s, oc, stats2)

            def flush_cs(s):
                oc, stats2, held = state[s].pop("dcs")
                for pr, ps in held:
                    for h in range(2):
                        nc.vector.bn_stats(stats2[:, 2 * pr + h, :], ps[:, h, :])
                        load["D"] += 660.0
                _cs_tail(s, oc, stats2)

            def _cs_tail(s, oc, stats2):
                mv2 = small.tile([128, 2], F32, tag="mv2", name="mv2")
                nc.vector.bn_aggr(mv2[:], stats2[:])
                load["D"] += 300.0
                rstd2, sig2 = rstd_chain(mv2, tag=f"c{oc}", want_sigma=True)
                negmr2 = small.tile([128, 1], F32, tag=f"negmr2_{oc}", name="negmr2")
                nc.vector.tensor_scalar(
                    negmr2[:], mv2[:, 0:1], rstd2[:], -1.0,
                    op0=AL.mult, op1=AL.mult,
                )
                diag2 = small.tile([128, 128], F16, tag=f"diag2_{oc}", name="diag2")
                nc.vector.tensor_scalar_mul(diag2[:], ident32[:], sig2[:])
                load["D"] += 400.0
                state[s].setdefault("cstats", {})[oc] = (rstd2, negmr2, diag2)

            def conv_out(s, oc, prs, alt=False):
                """Recompute conv chunks + diag residual, evict normalized f32,
                DMA out."""
                Z = state[s]["Z"]
                rstd2, negmr2, diag2 = state[s]["cstats"][oc]
                for pr in prs:
                    x_sb = state[s]["x"][pr]
                    ps = banks.tile([128, 2, 512], F32, tag="bank", name="ops")
                    for h in range(2):
                        cols = slice(1024 * pr + 512 * h, 1024 * pr + 512 * (h + 1))
                        xcols = slice(512 * h, 512 * (h + 1))
                        nc.tensor.matmul(
                            ps[:, h, :], ww_sb[:, 128 * oc:128 * (oc + 1)],
                            Z[:, cols], start=True, stop=False,
                        )
                        nc.tensor.matmul(
                            ps[:, h, :], diag2[:], x_sb[:, oc, xcols],
                            start=False, stop=True,
                        )
                    oring = orp.tile([128, 2, 512], F16, tag="oring", name="oring")
                    evict(oring[:], ps[:], EV_TS,
                          func="ts", bias=negmr2[:], scale=rstd2[:],
                          eng=("D" if pr % 2 else "A") if alt else None)
                    cols = slice(1024 * pr, 1024 * (pr + 1))
                    nc.sync.dma_start(
                        out_ext[s, 128 * oc:128 * (oc + 1), cols], oring[:],
                    )

            # ================= pipelined emission =================
            # PE queue stays dense: sample 1's proj matmuls run during
            # sample 0's finish/attn windows so HAM never sees an idle
            # MID window until the very tail.
            load_x(0)
            warmup()
            load_x(1)
            for pr in range(NPAIR):
                proj_pair(0, pr)
            proj_finish(0, 0)
            proj_finish(0, 1)
            proj_pair(1, 0)
            proj_finish(0, 2)
            proj_pair(1, 1)
            tn(0, 0)
            proj_pair(1, 2)
            tn(0, 1)
            gram(0)
            proj_pair(1, 3, defer=True)
            softmax(0)
            attn_transpose(0)
            flush_pp(1)
            zstage(0)
            proj_finish(1, 0)
            proj_finish(1, 1)
            proj_finish(1, 2)
            tn(1, 0)
            tn(1, 1)
            gram(1)
            conv_stats(0, 0, defer=True)
            softmax(1)
            attn_transpose(1)
            flush_cs(0)
            zstage(1)
            conv_stats(0, 1)
            conv_out(0, 0, range(NPAIR))
            conv_stats(1, 0)
            conv_out(0, 1, range(NPAIR))
            conv_stats(1, 1)
            conv_out(1, 0, range(NPAIR), alt=True)
            conv_out(1, 1, range(NPAIR), alt=True)

    nc.compile()
    return nc


def _get_nc():
    if "nc" not in _CACHE:
        _CACHE["nc"] = build_nc()
    return _CACHE["nc"]


def _prep_in_maps(x, g_w, theta_w, phi_w, W_w):
    # stacked projection lhsT: [c, ci] chunks -> [128, KCH, 3, 128]
    w3 = np.stack(
        [theta_w.T.reshape(KCH, 128, CI), phi_w.T.reshape(KCH, 128, CI),
         g_w.T.reshape(KCH, 128, CI)],
        axis=2,
    )  # [KCH, 128, 3, 128]
    w3 = np.ascontiguousarray(w3.transpose(1, 0, 2, 3)).astype(np.float16)
    ww = np.ascontiguousarray(W_w.T).astype(np.float16)  # [CI, C]
    xr = np.ascontiguousarray(x.reshape(B, C, N)).astype(np.float16)
    in_maps = []
    for c in range(N_CORES):
        in_maps.append({
            "x": xr[B_LOC * c:B_LOC * (c + 1)],
            "w3": w3,
            "ww": ww,
        })
    return in_maps


def kernel(x, g_w, g_b, theta_w, theta_b, phi_w, phi_b, W_w, W_b, **_ignored):
    # biases are mathematically dropped by the InstanceNorms
    nc = _get_nc()
    in_maps = _prep_in_maps(x, g_w, theta_w, phi_w, W_w)
    res = run_bass_kernel_spmd(nc, in_maps, core_ids=list(range(N_CORES)))
    outs = [res.results[c]["out"].reshape(B_LOC, C, H, W) for c in range(N_CORES)]
    return np.concatenate(outs, axis=0).astype(np.float32)


def _install_ntff_hook():
    """Provide antenv.axon_hooks if the image lacks it (see trn_boot.py)."""
    import types
    try:
        from antenv.axon_hooks import get_axon_ntff_profile_hook  # noqa: F401
        return
    except ImportError:
        pass
    import contextlib
    import ctypes

    so_path = "/opt/axon/libaxon_pjrt.so"
    lib = ctypes.CDLL(so_path)
    if not hasattr(lib, "axon_start_nrt_profile"):
        hook = None
    else:
        lib.axon_start_nrt_profile.argtypes = [
            ctypes.POINTER(ctypes.c_int64), ctypes.c_size_t]
        lib.axon_start_nrt_profile.restype = ctypes.c_int64
        lib.axon_stop_nrt_profile.argtypes = [ctypes.c_char_p]
        lib.axon_stop_nrt_profile.restype = ctypes.c_int64

        @contextlib.contextmanager
        def hook(output_dir, device_ids):
            import jax
            jax.devices()
            if device_ids:
                ids = (ctypes.c_int64 * len(device_ids))(*device_ids)
                rc = lib.axon_start_nrt_profile(ids, len(device_ids))
            else:
                rc = lib.axon_start_nrt_profile(None, 0)
            if rc != 0:
                raise RuntimeError(f"axon_start_nrt_profile rc={rc}")
            try:
                yield
            finally:
                n = lib.axon_stop_nrt_profile(str(output_dir).encode())
                if n <= 0:
                    raise RuntimeError(f"axon_stop_nrt_profile rc={n}")

    mod = types.ModuleType("antenv.axon_hooks")
    mod.get_axon_ntff_profile_hook = lambda: hook
    mod.set_axon_ntff_profile_hook = lambda h: None
    sys.modules["antenv.axon_hooks"] = mod


def run_traced(x, g_w, g_b, theta_w, theta_b, phi_w, phi_b, W_w, W_b, **_ignored):
    """Like kernel() but with NTFF profiling; returns (out, BassKernelResults)."""
    _install_ntff_hook()
    nc = _get_nc()
    in_maps = _prep_in_maps(x, g_w, theta_w, phi_w, W_w)
    res = run_bass_kernel_spmd(
        nc, in_maps, core_ids=list(range(N_CORES)), trace=True
    )
    outs = [res.results[c]["out"].reshape(B_LOC, C, H, W) for c in range(N_CORES)]
    return np.concatenate(outs, axis=0).astype(np.float32), res
